# revision 47
# baseline (speedup 1.0000x reference)
"""Trainium2 Bass kernel for BioBERT-ARG-GNN (gated pooling + 2-layer GCN + MLP head).

Strategy: pure data parallel over batch B=64 across 8 NeuronCores (8
graphs per core).  Host precomputes index-derived structures and ships
everything fp8(e4m3): a TRANSPOSED copy of last_hidden (lhT,
[hidden, tokens]), the one-hot pooling matrix P'' (with 1/cnt, D^-1/2
and a x16 fp8-range scale folded in) and the normalized adjacency
Ahat'' = 16 * D^-1/2 (A+I) D^-1/2.  The transposed lhT layout lets
BOTH the gate logits and the W1 projection run on the PE with the
contraction over the hidden dim, in fp8 DoubleRow mode (256-deep
contraction, 2x bf16 throughput):

    yT[gh, t] = sum_j W1c^T  @ lhT_j    (3 DR matmuls, free=512)
    lg[:, t]  = sum_j wrbc^T @ lhT_j    (3 DR matmuls; stationary =
                                         wr chunk broadcast to 128
                                         columns, so lg is born
                                         partition-broadcast)

W1 and wr are scaled by SC_W=64 so fp8 stays in the normal range; the
sigmoid un-scales via the ACT scale operand, and the P''/Ahat'' scales
cancel through W2 (/SC_P/SC_A/SC_W) and the mean column (/SC_A) —
scales pass through the relus, biases are folded on the host.  fp8
accuracy is essentially free here: measured rel-err 4.34e-3 vs 4.32e-3
for all-bf16 (threshold 2e-2).

sigmoid(lg) on ACT -> [128, 512] gates; DVE multiplies them into yT
(bf16); ONE DMA-XBAR transpose per graph PAIR turns ygT into
token-major y chunks; pooling contracts tokens directly:
t1 = P''^T (g*y) = pool(gated lh) @ W1.  GCN layer 1 is computed
TRANSPOSED (z1T = t1^T @ Ahat'', born feature-major) so layer 2 needs
no explicit transpose stage; mixed fp8-x-bf16 matmul operands are used
throughout.  The FC head is batched over all 8 graphs (cls ships
pre-transposed; the [2, BL] output is transposed back on the host).

Scheduling (all hard-won against the HWDGE realities: 8 rotating DMA
semaphore slots whose reuse-guards block the issuing engine queue, a
~400GB/s two-ring HBM budget that ramps from ~200GB/s, scheduler
wait-coalescing that proxies data deps onto "later" DMA completions,
and in-order engine queues):
 - ONE fp8 mega-transfer per graph (lhT + P'' + Ahat''), m0 split in
   two pieces so the first projection starts during the DMA ramp;
   megs alternate rings (sync: 0,1,4,6 / ACT: consts, 2,3,5,7).
 - consts ship in two pieces: the small early part (fp8 W1/wrb + W2/
   MEAN/IDENT) first on the ACT ring; the fat head consts (Wf1/Wf2/
   cls) ride the sync ring in its idle window between XBAR2 and XBAR3.
 - all XBAR transposes on the sync ring, whose bulk drains before the
   first XBAR's data is ready, so they clear in data-ready order.
 - a single software pipeline: step s issues graph s's projection
   matmuls while pooling + GCN stages for earlier graph pairs fill the
   PE slack; psL/psY double-buffered so projections never wait on the
   sigmoid/gate chain.
"""

import os
import sys

import numpy as np

for _p in ("/opt/trn_rl_repo", "/root/.axon_site/_ro/trn_rl_repo"):
    if os.path.isdir(_p) and _p not in sys.path:
        sys.path.insert(0, _p)

import ml_dtypes  # noqa: E402
import concourse.bass as bass  # noqa: E402
import concourse.mybir as mybir  # noqa: E402
from concourse import tile  # noqa: E402
from concourse.bass_utils import run_bass_kernel_spmd  # noqa: E402

# Problem shapes (hardcoded per contest rules).
B, S, H = 64, 512, 768
N, E = 128, 1024
GH, FH, L = 128, 256, 2
NCORES = 8
BL = B // NCORES  # graphs per core
SC = S // 128     # subtoken chunks per graph
HC = H // 128     # BERT-hidden chunks
FC = (H + GH) // 128  # concat-feature chunks for the FC head
SC_W = 64.0       # fp8 weight scale (W1, wr); folded back via P'/sigmoid

# fp8 consts column offsets
C8_W1 = 0                 # [HC*GH] = 768: [p, hc, j] = SC_W*W1[hc*128+p, j]
C8_WRB = HC * GH          # [HC*128]: [p, hc, m] = SC_W*wr[hc*128+p]
C8_W = C8_WRB + HC * 128  # 1536

# merged per-graph mega tensor (all fp8: lhT + scaled P'' + scaled Ahat'')
MEGW8 = HC * S + (SC + 1) * N  # 3072 + 640 = 3712
SC_P = 16.0   # fp8 scale on P' (undone via W2)
SC_A = 16.0   # fp8 scale on Ahat (undone via W2 and the mean column)

# bf16 consts column offsets: early block (GCN) first, head block after
C_W2 = 0                      # [GH]
C_MEAN = C_W2 + GH            # [1]
C_IDENT = C_MEAN + 8          # [128]
C_E = C_IDENT + 128           # 264 = early-consts boundary
C_WF1 = C_E                   # [FC*2*128] = 1792
C_WF2 = C_WF1 + FC * 2 * 128  # [2*L] = 4
C_CLS = C_WF2 + 2 * L         # [HC*BL] = 48
C_W = C_CLS + HC * BL

f32 = mybir.dt.float32
bf16 = mybir.dt.bfloat16
fp8 = mybir.dt.float8e4
AFT = mybir.ActivationFunctionType
ALU = mybir.AluOpType
MPM = mybir.MatmulPerfMode
BF16 = ml_dtypes.bfloat16
E4M3 = ml_dtypes.float8_e4m3

_CACHE = {}


def _split_multi_waits(nc: bass.Bass) -> int:
    """Walrus in this container accepts one sync-wait per instruction; split
    extra waits into single-wait EventSemaphore nops just before it."""
    n_split = 0
    for fn in nc.m.functions:
        for blk in fn.blocks:
            new_instrs = []
            changed = False
            for inst in blk.instructions:
                si = getattr(inst, "sync_info", None)
                if si is not None and si.on_wait is not None and len(si.on_wait) > 1:
                    waits = list(si.on_wait)
                    for j, w in enumerate(waits[:-1]):
                        ev = mybir.InstEventSemaphore(
                            name=f"{inst.name}_ws{j}",
                            ins=[], outs=[],
                            engine=inst.engine,
                            sync_info=mybir.SyncInfo(on_wait=[w], on_update=[]),
                        )
                        new_instrs.append(ev)
                    inst.sync_info = mybir.SyncInfo(
                        on_wait=[waits[-1]], on_update=list(si.on_update))
                    n_split += 1
                    changed = True
                new_instrs.append(inst)
            if changed:
                blk.instructions = new_instrs
    return n_split


def build_program(br_val: float, b1_zero: bool, b2_zero: bool,
                  bf1_zero: bool, bf2_zero: bool) -> bass.Bass:
    nc = bass.Bass()

    meg8_d = nc.declare_dram_parameter("meg8", [BL, 128, MEGW8], fp8,
                                       isOutput=False)
    cts_d = nc.declare_dram_parameter("cts", [128, C8_W + 2 * C_W], fp8,
                                      isOutput=False)
    b1b_d = nc.declare_dram_parameter("b1b", [128, GH], f32, isOutput=False)
    b2b_d = nc.declare_dram_parameter("b2b", [128, GH], f32, isOutput=False)
    bf1b_d = nc.declare_dram_parameter("bf1b", [128, 2], f32, isOutput=False)
    bf2b_d = nc.declare_dram_parameter("bf2b", [L, 1], f32, isOutput=False)
    out_d = nc.declare_dram_parameter("out", [L, BL], f32, isOutput=True)

    with tile.TileContext(nc) as tc:
        with (
            tc.tile_pool(name="const", bufs=1) as cpool,
            tc.tile_pool(name="megp", bufs=BL) as megpool,
            tc.tile_pool(name="work", bufs=3) as wpool,
            tc.tile_pool(name="psY", bufs=2, space="PSUM") as psY,
            tc.tile_pool(name="psL", bufs=2, space="PSUM") as psL,
            tc.tile_pool(name="psB", bufs=4, space="PSUM") as psB,
        ):
            # consts ride the ACT ring first — but only the parts needed
            # before the head (fp8 W1/wrb + W2/MEAN/IDENT, ~250KB).  The fat
            # head consts (Wf1/Wf2/cls) ship in a separate transfer emitted
            # after the pipeline loop: they are needed only at ~35us and
            # this keeps early HBM bandwidth for m0.
            cts = cpool.tile([128, C8_W + 2 * C_W], fp8)
            nc.scalar.dma_start(cts[:, 0:C8_W + 2 * C_E],
                                cts_d[:, 0:C8_W + 2 * C_E])
            ct8 = cts[:, 0:C8_W].rearrange("p (c g) -> p c g", c=HC)
            ctB = cts[:, C8_W:].bitcast(bf16)
            b1t = b2t = bf1t = bf2t = None
            if not b1_zero:
                b1t = cpool.tile([128, GH], f32, name="b1t")
                nc.scalar.dma_start(b1t[:], b1b_d[:])
            if not b2_zero:
                b2t = cpool.tile([128, GH], f32, name="b2t")
                nc.scalar.dma_start(b2t[:], b2b_d[:])
            if not bf1_zero:
                bf1t = cpool.tile([128, 2], f32, name="bf1t")
                nc.scalar.dma_start(bf1t[:], bf1b_d[:])
            if not bf2_zero:
                bf2t = cpool.tile([L, 1], f32, name="bf2t")
                nc.scalar.dma_start(bf2t[:], bf2b_d[:])
            catT6 = cpool.tile([128, BL], bf16)
            h1r = cpool.tile([128, 2, BL], bf16)

            # meg delivery: one fp8 transfer per graph (lhT + P'' + Ahat''
            # in one blob) in consumption order, split over the two HWDGE
            # rings.  Few, large transfers matter: only 8 DMA semaphore
            # slots exist, their reuse-guard waits block the issuing engine
            # queue, and the tile scheduler proxies data deps onto DMA
            # completions it models as "later" — so every transfer must
            # complete before its slot's second user needs to launch.
            megs8 = []   # [128, HC, S] fp8 lhT views
            megsb = []   # [128, (SC+1)*N] fp8 views (P'' chunks + Ahat'')
            mtiles = []
            for g in range(BL):
                m = megpool.tile([128, MEGW8], fp8, tag=f"m{g}", bufs=1,
                                 name=f"m{g}")
                mtiles.append(m)
                megs8.append(m[:, 0:HC * S].rearrange(
                    "p (c s) -> p c s", c=HC))
                megsb.append(m[:, HC * S:MEGW8])
            # m0 in two pieces on the sync ring: the first two DR chunks
            # (256KB) land ~1.3us before the rest, letting the first
            # projection start while the DMA engines still ramp.
            nc.sync.dma_start(mtiles[0][:, 0:4 * S], meg8_d[0][:, 0:4 * S])
            nc.sync.dma_start(mtiles[0][:, 4 * S:MEGW8],
                              meg8_d[0][:, 4 * S:MEGW8])
            SYNC_MEGS = (1, 4, 6)
            for g in range(1, BL):
                eng = nc.sync if g in SYNC_MEGS else nc.scalar
                eng.dma_start(mtiles[g][:], meg8_d[g])

            W1c = ct8[:, :, 0:GH]          # [128, HC, GH]
            WRBc = ct8[:, :, GH:GH + 128]  # [128, HC, 128]

            W2c = ctB[:, C_W2:C_W2 + GH]
            MEAN = ctB[:, C_MEAN:C_MEAN + 1]
            IDENT = ctB[:, C_IDENT:C_IDENT + 128]

            yT_ps = [None] * BL
            lg_ps = [None] * BL
            gate_sb = [None] * BL
            y_sb = [None] * BL
            t1sb = [None] * BL
            x1 = [None] * BL
            t2sb = [None] * BL
            x2 = [None] * BL

            def relu_to(out_sb, z_ps, bias_tile, tag, bias_col=False):
                if bias_tile is None:
                    nc.vector.tensor_scalar_max(out_sb[:], z_ps[:], 0.0)
                elif bias_col:
                    nc.vector.tensor_scalar(out_sb[:], z_ps[:],
                                            bias_tile[:, 0:1], 0.0,
                                            ALU.add, ALU.max)
                else:
                    tmp = wpool.tile([128, GH], f32, tag=tag + "b", bufs=2,
                                     name=tag + "b")
                    nc.vector.tensor_tensor(tmp[:], z_ps[:], bias_tile[:],
                                            ALU.add)
                    nc.vector.tensor_scalar_max(out_sb[:], tmp[:], 0.0)

            ygsb2 = [None] * (BL // 2)
            y2 = [None] * (BL // 2)

            def gate_into_y(g):
                """multiply broadcast sigmoid gates into yT, XBAR per pair."""
                k, half = g // 2, g % 2
                if half == 0:
                    ygsb2[k] = wpool.tile([128, 2, S], bf16, tag="ygsb",
                                          bufs=4, name="ygsb")
                nc.vector.tensor_tensor(ygsb2[k][:, half, :], yT_ps[g][:],
                                        gate_sb[g][:], ALU.mult)
                if half == 1:
                    y2[k] = wpool.tile([128, 2 * SC, 128], bf16, tag="ysb",
                                       bufs=BL // 2, name="y_sb")
                    # all XBAR transposes on the sync ring (its meg bulk
                    # drains early), clearing in data-ready order
                    nc.sync.dma_start(y2[k][:], ygsb2[k][:], transpose=True)
                    y_sb[2 * k] = y2[k][:, 0:SC, :]
                    y_sb[2 * k + 1] = y2[k][:, SC:2 * SC, :]

            def pool_g(g):
                t1_ps = psB.tile([128, GH], f32, tag="mm", name="t1_ps")
                for c in range(SC):
                    nc.tensor.matmul(
                        t1_ps[:],
                        megsb[g][:, c * N:(c + 1) * N],
                        y_sb[g][:, c, :], start=(c == 0), stop=(c == SC - 1))
                t1sb[g] = wpool.tile([128, GH], bf16, tag="t1sb", bufs=BL,
                                     name="t1sb")
                nc.vector.tensor_copy(t1sb[g][:], t1_ps[:])

            def z1_g(g):
                # transposed GCN layer 1: z1T = t1^T @ Ahat — born
                # feature-major, so layer 2 needs no explicit transpose
                z_ps = psB.tile([128, GH], f32, tag="mm", name="z_ps")
                nc.tensor.matmul(z_ps[:], t1sb[g][:],
                                 megsb[g][:, SC * N:(SC + 1) * N],
                                 start=True, stop=True)
                x1[g] = wpool.tile([128, GH], bf16, tag="x1", bufs=BL,
                                   name="x1")
                relu_to(x1[g], z_ps, b1t, "x1", bias_col=True)

            def w2_g(g):
                t2_ps = psB.tile([128, GH], f32, tag="mm", name="t2_ps")
                nc.tensor.matmul(t2_ps[:], x1[g][:], W2c,
                                 start=True, stop=True)
                t2sb[g] = wpool.tile([128, GH], bf16, tag="t2sb", bufs=BL,
                                     name="t2sb")
                nc.scalar.copy(t2sb[g][:], t2_ps[:])

            def z2_g(g):
                z2_ps = psB.tile([128, GH], f32, tag="mm", name="z2_ps")
                nc.tensor.matmul(z2_ps[:], megsb[g][:, SC * N:(SC + 1) * N],
                                 t2sb[g][:], start=True, stop=True)
                x2[g] = wpool.tile([128, GH], bf16, tag="x2", bufs=BL,
                                   name="x2")
                relu_to(x2[g], z2_ps, b2t, "x2")

            def mp_g(g):
                mp_ps = psB.tile([128, 1], f32, tag="mm", name="mp_ps")
                nc.tensor.matmul(mp_ps[:], x2[g][:], MEAN,
                                 start=True, stop=True)
                nc.vector.tensor_copy(catT6[:, g:g + 1], mp_ps[:])

            # ---- unified software pipeline: projection/gate (steps 0..8)
            # with pooling+GCN stages for finished pairs filling PE slack ----
            STAGES = [pool_g, z1_g, w2_g, z2_g, mp_g]
            for s in range(BL + 7):
                # deep stages first (their data has been ready the longest);
                # pool (stage 0) last — its XBAR data is the freshest.
                for si in range(len(STAGES) - 1, 0, -1):
                    k2 = s - 3 - si  # == 2*k when this pair's stage is due
                    if k2 >= 0 and k2 % 2 == 0 and k2 < BL:
                        STAGES[si](k2)
                        STAGES[si](k2 + 1)
                if s < BL:
                    if s >= 1:
                        gate_sb[s - 1] = wpool.tile([128, S], bf16, tag="gt",
                                                    bufs=4, name="gate_sb")
                        nc.scalar.activation(gate_sb[s - 1][:],
                                             lg_ps[s - 1][:], AFT.Sigmoid,
                                             bias=float(br_val),
                                             scale=1.0 / SC_W)
                    yT_ps[s] = psY.tile([128, S], f32, tag="yt", name="yT_ps")
                    lg_ps[s] = psL.tile([128, S], f32, tag="lg", name="lg_ps")
                    for j in range(HC // 2):
                        lht_j = megs8[s][:, 2 * j:2 * j + 2, :]
                        nc.tensor.matmul(
                            yT_ps[s][:], W1c[:, 2 * j:2 * j + 2, :],
                            lht_j, start=(j == 0), stop=(j == HC // 2 - 1),
                            perf_mode=MPM.DoubleRow)
                        nc.tensor.matmul(
                            lg_ps[s][:], WRBc[:, 2 * j:2 * j + 2, :],
                            lht_j, start=(j == 0), stop=(j == HC // 2 - 1),
                            perf_mode=MPM.DoubleRow)
                    if s >= 1:
                        gate_into_y(s - 1)
                if s == BL:
                    gate_sb[BL - 1] = wpool.tile([128, S], bf16, tag="gt",
                                                 bufs=4, name="gate_sb")
                    nc.scalar.activation(gate_sb[BL - 1][:],
                                         lg_ps[BL - 1][:], AFT.Sigmoid,
                                         bias=float(br_val),
                                         scale=1.0 / SC_W)
                    gate_into_y(BL - 1)
                if s == 7:
                    # late head consts (Wf1/Wf2/cls): ride the sync ring in
                    # its idle window between XBAR2 and XBAR3, far from the
                    # scalar ring's meg stream
                    nc.sync.dma_start(cts[:, C8_W + 2 * C_E:],
                                      cts_d[:, C8_W + 2 * C_E:])
                if s >= 3 and (s - 3) % 2 == 0 and s - 3 < BL:
                    pool_g(s - 3)
                    pool_g(s - 2)

            # ---------- FC head over all BL graphs ----------
            h1_ps = []
            for hh in range(2):
                hp = psB.tile([128, BL], f32, tag="mm", name=f"h1_ps{hh}")
                for c in range(FC):
                    lhsT = ctB[:, C_WF1 + (c * 2 + hh) * 128:
                               C_WF1 + (c * 2 + hh + 1) * 128]
                    rhs = (ctB[:, C_CLS + c * BL:C_CLS + (c + 1) * BL]
                           if c < HC else catT6[:])
                    nc.tensor.matmul(hp[:], lhsT, rhs, start=(c == 0),
                                     stop=(c == FC - 1))
                h1_ps.append(hp)
            for hh in range(2):
                if bf1t is None:
                    nc.vector.tensor_scalar_max(h1r[:, hh, :], h1_ps[hh][:],
                                                0.0)
                else:
                    nc.vector.tensor_scalar(h1r[:, hh, :], h1_ps[hh][:],
                                            bf1t[:, hh:hh + 1], 0.0,
                                            ALU.add, ALU.max)
            out_ps = psB.tile([L, BL], f32, tag="mm", name="out_ps")
            for hh in range(2):
                nc.tensor.matmul(out_ps[:],
                                 ctB[:, C_WF2 + hh * L:C_WF2 + (hh + 1) * L],
                                 h1r[:, hh, :], start=(hh == 0),
                                 stop=(hh == 1))
            outs = cpool.tile([L, BL], f32)
            if bf2t is None:
                nc.vector.tensor_copy(outs[:], out_ps[:])
            else:
                nc.vector.tensor_scalar_add(outs[:], out_ps[:], bf2t[:])
            nc.sync.dma_start(out_d[:], outs[:])

    _split_multi_waits(nc)
    return nc


def _prepare_in_maps(inputs):
    lh = np.ascontiguousarray(np.asarray(inputs["last_hidden"], dtype=np.float32))
    submap = np.asarray(inputs["submap"]).astype(np.int64)
    edge_index = np.asarray(inputs["edge_index"]).astype(np.int64)
    assert lh.shape == (B, S, H)
    assert int(inputs.get("num_nodes", N)) == N

    wr = np.asarray(inputs["wr"], dtype=np.float32)
    br = float(np.asarray(inputs["br"], dtype=np.float32))
    W1 = np.asarray(inputs["W1"], dtype=np.float32)
    b1 = np.asarray(inputs["b1"], dtype=np.float32)
    W2 = np.asarray(inputs["W2"], dtype=np.float32)
    b2 = np.asarray(inputs["b2"], dtype=np.float32)
    Wf1 = np.asarray(inputs["Wf1"], dtype=np.float32)
    bf1 = np.asarray(inputs["bf1"], dtype=np.float32)
    Wf2 = np.asarray(inputs["Wf2"], dtype=np.float32)
    bf2 = np.asarray(inputs["bf2"], dtype=np.float32)

    # ---- host-side index prep: adjacency, degrees, counts ----
    src = edge_index[:, 0, :]
    dst = edge_index[:, 1, :]
    flat = (np.arange(B, dtype=np.int64)[:, None] * (N * N) + src * N + dst)
    A = np.bincount(flat.reshape(-1), minlength=B * N * N).astype(np.float32)
    A = A.reshape(B, N, N) + np.eye(N, dtype=np.float32)[None]
    deg = A.sum(axis=1)                      # in-degree incl self-loops
    dinv = 1.0 / np.sqrt(deg)
    ahat = A * dinv[:, :, None] * dinv[:, None, :]

    cflat = np.arange(B, dtype=np.int64)[:, None] * N + submap
    cnt = np.bincount(cflat.reshape(-1), minlength=B * N).astype(np.float32)
    invc = 1.0 / np.maximum(cnt.reshape(B, N), 1.0)

    P = (submap[:, :, None] == np.arange(N)[None, None, :]).astype(np.float32)
    P *= (invc * dinv)[:, None, :] * SC_P

    # ---- merged mega-tensor: all fp8 (lhT + scaled P'' + scaled Ahat'') ----
    lht = np.ascontiguousarray(
        lh.astype(E4M3).reshape(B, S, HC, 128).transpose(0, 3, 2, 1))
    p_r = P.astype(E4M3).reshape(B, SC, 128, N).transpose(0, 2, 1, 3)
    meg8 = np.empty((B, 128, MEGW8), dtype=E4M3)
    meg8[:, :, 0:HC * S] = lht.reshape(B, 128, HC * S)
    meg8[:, :, HC * S:HC * S + SC * N] = p_r.reshape(B, 128, SC * N)
    meg8[:, :, HC * S + SC * N:] = (SC_A * ahat).astype(E4M3)

    # ---- fp8 consts: scaled W1 + broadcast wr ----
    ct8 = np.zeros((128, C8_W), dtype=np.float32)
    ct8[:, C8_W1:C8_W1 + HC * GH] = (
        (SC_W * W1).reshape(HC, 128, GH).transpose(1, 0, 2).reshape(128, -1))
    ct8[:, C8_WRB:C8_WRB + HC * 128] = np.repeat(
        (SC_W * wr).reshape(HC, 128).T, 128, axis=1).reshape(128, HC * 128)
    # interleave into [p, hc, GH+128] layout matching the SBUF tile
    ct8_t = np.empty((128, HC, GH + 128), dtype=E4M3)
    ct8_t[:, :, 0:GH] = ct8[:, 0:HC * GH].reshape(128, HC, GH).astype(E4M3)
    ct8_t[:, :, GH:] = ct8[:, HC * GH:].reshape(128, HC, 128).astype(E4M3)
    ct8_flat = np.ascontiguousarray(ct8_t.reshape(128, -1))

    # ---- bf16 consts, cls block differs per core ----
    consts = np.zeros((128, C_W), dtype=np.float32)
    consts[:, C_W2:C_W2 + GH] = W2 / (SC_P * SC_A * SC_W)
    consts[:, C_WF1:C_WF1 + FC * 2 * 128] = (
        Wf1.reshape(FC, 128, 2, 128).transpose(1, 0, 2, 3).reshape(128, -1))
    consts[:, C_WF2:C_WF2 + 2 * L] = (
        Wf2.reshape(2, 128, L).transpose(1, 0, 2).reshape(128, 2 * L))
    consts[:, C_MEAN] = 1.0 / (SC_A * N)
    consts[:, C_IDENT:C_IDENT + 128] = np.eye(128, dtype=np.float32)

    b1b = np.ascontiguousarray(np.broadcast_to(
        (b1 * (SC_P * SC_A * SC_W)).reshape(GH, 1), (128, GH))
        .astype(np.float32))
    b2b = np.ascontiguousarray(np.broadcast_to(
        b2 * SC_A, (128, GH)).astype(np.float32))
    bf1b = np.ascontiguousarray(bf1.reshape(2, 128).T.astype(np.float32))
    bf2b = np.ascontiguousarray(bf2.reshape(L, 1).astype(np.float32))

    in_maps = []
    for i in range(NCORES):
        sl = slice(i * BL, (i + 1) * BL)
        ci = consts.copy()
        ci[:, C_CLS:C_CLS + HC * BL] = (
            lh[sl, 0, :].reshape(BL, HC, 128).transpose(2, 1, 0)
            .reshape(128, HC * BL))
        cb = ci.astype(BF16)
        cts = np.empty((128, C8_W + 2 * C_W), dtype=np.uint8)
        cts[:, 0:C8_W] = ct8_flat.view(np.uint8)
        cts[:, C8_W:] = cb.view(np.uint8)
        in_maps.append({
            "meg8": np.ascontiguousarray(meg8[sl]),
            "cts": cts.view(E4M3),
            "b1b": b1b, "b2b": b2b, "bf1b": bf1b, "bf2b": bf2b,
        })
    flags = (br, bool(np.all(b1 == 0)), bool(np.all(b2 == 0)),
             bool(np.all(bf1 == 0)), bool(np.all(bf2 == 0)))
    return in_maps, flags


def _run(inputs, trace=False):
    in_maps, flags = _prepare_in_maps(inputs)
    key = ("prog",) + flags
    if key not in _CACHE:
        _CACHE[key] = build_program(*flags)
    nc = _CACHE[key]
    res = run_bass_kernel_spmd(nc, in_maps, list(range(NCORES)), trace=trace)
    out = np.concatenate(
        [np.asarray(res.results[i]["out"]).T for i in range(NCORES)],
        axis=0).astype(np.float32)
    return out, res


def kernel(**inputs) -> np.ndarray:
    out, _ = _run(inputs, trace=False)
    return out


# revision 51
# speedup vs baseline: 1.0067x; 1.0067x over previous
"""Trainium2 Bass kernel for BioBERT-ARG-GNN (gated pooling + 2-layer GCN + MLP head).

Strategy: pure data parallel over batch B=64 across 8 NeuronCores (8
graphs per core).  Host precomputes index-derived structures and ships
everything fp8(e4m3): a TRANSPOSED copy of last_hidden (lhT,
[hidden, tokens]), the one-hot pooling matrix P'' (with 1/cnt, D^-1/2
and a x16 fp8-range scale folded in) and the normalized adjacency
Ahat'' = 16 * D^-1/2 (A+I) D^-1/2.  The transposed lhT layout lets
BOTH the gate logits and the W1 projection run on the PE with the
contraction over the hidden dim, in fp8 DoubleRow mode (256-deep
contraction, 2x bf16 throughput):

    yT[gh, t] = sum_j W1c^T  @ lhT_j    (3 DR matmuls, free=512)
    lg[:, t]  = sum_j wrbc^T @ lhT_j    (3 DR matmuls; stationary =
                                         wr chunk broadcast to 128
                                         columns, so lg is born
                                         partition-broadcast)

W1 and wr are scaled by SC_W=64 so fp8 stays in the normal range; the
sigmoid un-scales via the ACT scale operand, and the P''/Ahat'' scales
cancel through W2 (/SC_P/SC_A/SC_W) and the mean column (/SC_A) —
scales pass through the relus, biases are folded on the host.  fp8
accuracy is essentially free here: measured rel-err 4.34e-3 vs 4.32e-3
for all-bf16 (threshold 2e-2).

sigmoid(lg) on ACT -> [128, 512] gates; DVE multiplies them into yT
(bf16); ONE DMA-XBAR transpose per graph PAIR turns ygT into
token-major y chunks; pooling contracts tokens directly:
t1 = P''^T (g*y) = pool(gated lh) @ W1.  GCN layer 1 is computed
TRANSPOSED (z1T = t1^T @ Ahat'', born feature-major) so layer 2 needs
no explicit transpose stage; mixed fp8-x-bf16 matmul operands are used
throughout.  The FC head is batched over all 8 graphs (cls ships
pre-transposed; the [2, BL] output is transposed back on the host).

Scheduling (all hard-won against the HWDGE realities: 8 rotating DMA
semaphore slots whose reuse-guards block the issuing engine queue, a
~400GB/s two-ring HBM budget that ramps from ~200GB/s, scheduler
wait-coalescing that proxies data deps onto "later" DMA completions,
and in-order engine queues):
 - ONE fp8 mega-transfer per graph (lhT + P'' + Ahat''), m0 split in
   two pieces so the first projection starts during the DMA ramp;
   megs alternate rings (sync: 0,1,4,6 / ACT: consts, 2,3,5,7).
 - consts ship in two pieces: the small early part (fp8 W1/wrb + W2/
   MEAN/IDENT) first on the ACT ring; the fat head consts (Wf1/Wf2/
   cls) ride the sync ring in its idle window between XBAR2 and XBAR3.
 - all XBAR transposes on the sync ring, whose bulk drains before the
   first XBAR's data is ready, so they clear in data-ready order.
 - a single software pipeline: step s issues graph s's projection
   matmuls while pooling + GCN stages for earlier graph pairs fill the
   PE slack; psL/psY double-buffered so projections never wait on the
   sigmoid/gate chain.
"""

import os
import sys

import numpy as np

for _p in ("/opt/trn_rl_repo", "/root/.axon_site/_ro/trn_rl_repo"):
    if os.path.isdir(_p) and _p not in sys.path:
        sys.path.insert(0, _p)

import ml_dtypes  # noqa: E402
import concourse.bass as bass  # noqa: E402
import concourse.mybir as mybir  # noqa: E402
from concourse import tile  # noqa: E402
from concourse.bass_utils import run_bass_kernel_spmd  # noqa: E402

# Problem shapes (hardcoded per contest rules).
B, S, H = 64, 512, 768
N, E = 128, 1024
GH, FH, L = 128, 256, 2
NCORES = 8
BL = B // NCORES  # graphs per core
SC = S // 128     # subtoken chunks per graph
HC = H // 128     # BERT-hidden chunks
FC = (H + GH) // 128  # concat-feature chunks for the FC head
SC_W = 64.0       # fp8 weight scale (W1, wr); folded back via P'/sigmoid

# fp8 consts column offsets
C8_W1 = 0                 # [HC*GH] = 768: [p, hc, j] = SC_W*W1[hc*128+p, j]
C8_WRB = HC * GH          # [HC*128]: [p, hc, m] = SC_W*wr[hc*128+p]
C8_W = C8_WRB + HC * 128  # 1536

# merged per-graph mega tensor (all fp8: lhT + scaled P'' + scaled Ahat'')
MEGW8 = HC * S + (SC + 1) * N  # 3072 + 640 = 3712
SC_P = 16.0   # fp8 scale on P' (undone via W2)
SC_A = 16.0   # fp8 scale on Ahat (undone via W2 and the mean column)

# bf16 consts column offsets: early block (GCN) first, head block after
C_W2 = 0                      # [GH]
C_MEAN = C_W2 + GH            # [1]
C_IDENT = C_MEAN + 8          # [128]
C_E = C_IDENT + 128           # 264 = early-consts boundary
C_WF1 = C_E                   # [FC*2*128] = 1792
C_WF2 = C_WF1 + FC * 2 * 128  # [2*L] = 4
C_CLS = C_WF2 + 2 * L         # [HC*BL] = 48
C_W = C_CLS + HC * BL

f32 = mybir.dt.float32
bf16 = mybir.dt.bfloat16
fp8 = mybir.dt.float8e4
AFT = mybir.ActivationFunctionType
ALU = mybir.AluOpType
MPM = mybir.MatmulPerfMode
BF16 = ml_dtypes.bfloat16
E4M3 = ml_dtypes.float8_e4m3

_CACHE = {}


def _split_multi_waits(nc: bass.Bass) -> int:
    """Walrus in this container accepts one sync-wait per instruction; split
    extra waits into single-wait EventSemaphore nops just before it."""
    n_split = 0
    for fn in nc.m.functions:
        for blk in fn.blocks:
            new_instrs = []
            changed = False
            for inst in blk.instructions:
                si = getattr(inst, "sync_info", None)
                if si is not None and si.on_wait is not None and len(si.on_wait) > 1:
                    waits = list(si.on_wait)
                    for j, w in enumerate(waits[:-1]):
                        ev = mybir.InstEventSemaphore(
                            name=f"{inst.name}_ws{j}",
                            ins=[], outs=[],
                            engine=inst.engine,
                            sync_info=mybir.SyncInfo(on_wait=[w], on_update=[]),
                        )
                        new_instrs.append(ev)
                    inst.sync_info = mybir.SyncInfo(
                        on_wait=[waits[-1]], on_update=list(si.on_update))
                    n_split += 1
                    changed = True
                new_instrs.append(inst)
            if changed:
                blk.instructions = new_instrs
    return n_split


def build_program(br_val: float, b1_zero: bool, b2_zero: bool,
                  bf1_zero: bool, bf2_zero: bool) -> bass.Bass:
    nc = bass.Bass()

    meg8_d = nc.declare_dram_parameter("meg8", [BL, 128, MEGW8], fp8,
                                       isOutput=False)
    cts_d = nc.declare_dram_parameter("cts", [128, C8_W + 2 * C_W], fp8,
                                      isOutput=False)
    b1b_d = nc.declare_dram_parameter("b1b", [128, GH], f32, isOutput=False)
    b2b_d = nc.declare_dram_parameter("b2b", [128, GH], f32, isOutput=False)
    bf1b_d = nc.declare_dram_parameter("bf1b", [128, 2], f32, isOutput=False)
    bf2b_d = nc.declare_dram_parameter("bf2b", [L, 1], f32, isOutput=False)
    out_d = nc.declare_dram_parameter("out", [L, BL], f32, isOutput=True)

    with tile.TileContext(nc) as tc:
        with (
            tc.tile_pool(name="const", bufs=1) as cpool,
            tc.tile_pool(name="megp", bufs=BL) as megpool,
            tc.tile_pool(name="work", bufs=3) as wpool,
            tc.tile_pool(name="psY", bufs=2, space="PSUM") as psY,
            tc.tile_pool(name="psL", bufs=2, space="PSUM") as psL,
            tc.tile_pool(name="psB", bufs=4, space="PSUM") as psB,
        ):
            # consts ride the ACT ring first — but only the parts needed
            # before the head (fp8 W1/wrb + W2/MEAN/IDENT, ~250KB).  The fat
            # head consts (Wf1/Wf2/cls) ship in a separate transfer emitted
            # after the pipeline loop: they are needed only at ~35us and
            # this keeps early HBM bandwidth for m0.
            cts = cpool.tile([128, C8_W + 2 * C_W], fp8)
            nc.scalar.dma_start(cts[:, 0:C8_W + 2 * C_E],
                                cts_d[:, 0:C8_W + 2 * C_E])
            ct8 = cts[:, 0:C8_W].rearrange("p (c g) -> p c g", c=HC)
            ctB = cts[:, C8_W:].bitcast(bf16)
            b1t = b2t = bf1t = bf2t = None
            if not b1_zero:
                b1t = cpool.tile([128, GH], f32, name="b1t")
                nc.scalar.dma_start(b1t[:], b1b_d[:])
            if not b2_zero:
                b2t = cpool.tile([128, GH], f32, name="b2t")
                nc.scalar.dma_start(b2t[:], b2b_d[:])
            if not bf1_zero:
                bf1t = cpool.tile([128, 2], f32, name="bf1t")
                nc.scalar.dma_start(bf1t[:], bf1b_d[:])
            if not bf2_zero:
                bf2t = cpool.tile([L, 1], f32, name="bf2t")
                nc.scalar.dma_start(bf2t[:], bf2b_d[:])
            catT6 = cpool.tile([128, BL], bf16)
            h1r = cpool.tile([128, 2, BL], bf16)

            # meg delivery: one fp8 transfer per graph (lhT + P'' + Ahat''
            # in one blob) in consumption order, split over the two HWDGE
            # rings.  Few, large transfers matter: only 8 DMA semaphore
            # slots exist, their reuse-guard waits block the issuing engine
            # queue, and the tile scheduler proxies data deps onto DMA
            # completions it models as "later" — so every transfer must
            # complete before its slot's second user needs to launch.
            megs8 = []   # [128, HC, S] fp8 lhT views
            megsb = []   # [128, (SC+1)*N] fp8 views (P'' chunks + Ahat'')
            mtiles = []
            for g in range(BL):
                m = megpool.tile([128, MEGW8], fp8, tag=f"m{g}", bufs=1,
                                 name=f"m{g}")
                mtiles.append(m)
                megs8.append(m[:, 0:HC * S].rearrange(
                    "p (c s) -> p c s", c=HC))
                megsb.append(m[:, HC * S:MEGW8])
            # m0 in two pieces on the sync ring: the first two DR chunks
            # (256KB) land ~1.3us before the rest, letting the first
            # projection start while the DMA engines still ramp.
            nc.sync.dma_start(mtiles[0][:, 0:4 * S], meg8_d[0][:, 0:4 * S])
            nc.sync.dma_start(mtiles[0][:, 4 * S:MEGW8],
                              meg8_d[0][:, 4 * S:MEGW8])

            # PE warmup: ~25 throwaway matmuls on a zeroed tile keep the
            # tensor engine's p-state ramped through the DMA head wait, so
            # the first real projections run at full clock instead of the
            # slow-ramp rate (first DR matmuls measured 634ns vs 376ns).
            wu = cpool.tile([128, S], bf16, name="warm")
            nc.gpsimd.memset(wu[:], 0)
            wu_ps = psB.tile([128, S], f32, tag="mm", name="wu_ps")
            for _ in range(10):
                nc.tensor.matmul(wu_ps[:], wu[:, 0:128], wu[:],
                                 start=True, stop=True)
            SYNC_MEGS = (2, 4, 6)
            for g in range(1, BL):
                eng = nc.sync if g in SYNC_MEGS else nc.scalar
                eng.dma_start(mtiles[g][:], meg8_d[g])

            W1c = ct8[:, :, 0:GH]          # [128, HC, GH]
            WRBc = ct8[:, :, GH:GH + 128]  # [128, HC, 128]

            W2c = ctB[:, C_W2:C_W2 + GH]
            MEAN = ctB[:, C_MEAN:C_MEAN + 1]
            IDENT = ctB[:, C_IDENT:C_IDENT + 128]

            yT_ps = [None] * BL
            lg_ps = [None] * BL
            gate_sb = [None] * BL
            y_sb = [None] * BL
            t1sb = [None] * BL
            x1 = [None] * BL
            t2sb = [None] * BL
            x2 = [None] * BL

            def relu_to(out_sb, z_ps, bias_tile, tag, bias_col=False):
                if bias_tile is None:
                    nc.vector.tensor_scalar_max(out_sb[:], z_ps[:], 0.0)
                elif bias_col:
                    nc.vector.tensor_scalar(out_sb[:], z_ps[:],
                                            bias_tile[:, 0:1], 0.0,
                                            ALU.add, ALU.max)
                else:
                    tmp = wpool.tile([128, GH], f32, tag=tag + "b", bufs=2,
                                     name=tag + "b")
                    nc.vector.tensor_tensor(tmp[:], z_ps[:], bias_tile[:],
                                            ALU.add)
                    nc.vector.tensor_scalar_max(out_sb[:], tmp[:], 0.0)

            ygsb2 = [None] * (BL // 2)
            y2 = [None] * (BL // 2)

            def gate_into_y(g):
                """multiply broadcast sigmoid gates into yT, XBAR per pair."""
                k, half = g // 2, g % 2
                if half == 0:
                    ygsb2[k] = wpool.tile([128, 2, S], bf16, tag="ygsb",
                                          bufs=4, name="ygsb")
                nc.vector.tensor_tensor(ygsb2[k][:, half, :], yT_ps[g][:],
                                        gate_sb[g][:], ALU.mult)
                if half == 1:
                    y2[k] = wpool.tile([128, 2 * SC, 128], bf16, tag="ysb",
                                       bufs=BL // 2, name="y_sb")
                    # all XBAR transposes on the sync ring (its meg bulk
                    # drains early), clearing in data-ready order
                    nc.sync.dma_start(y2[k][:], ygsb2[k][:], transpose=True)
                    y_sb[2 * k] = y2[k][:, 0:SC, :]
                    y_sb[2 * k + 1] = y2[k][:, SC:2 * SC, :]

            def pool_g(g):
                t1_ps = psB.tile([128, GH], f32, tag="mm", name="t1_ps")
                for c in range(SC):
                    nc.tensor.matmul(
                        t1_ps[:],
                        megsb[g][:, c * N:(c + 1) * N],
                        y_sb[g][:, c, :], start=(c == 0), stop=(c == SC - 1))
                t1sb[g] = wpool.tile([128, GH], bf16, tag="t1sb", bufs=BL,
                                     name="t1sb")
                nc.vector.tensor_copy(t1sb[g][:], t1_ps[:])

            def z1_g(g):
                # transposed GCN layer 1: z1T = t1^T @ Ahat — born
                # feature-major, so layer 2 needs no explicit transpose
                z_ps = psB.tile([128, GH], f32, tag="mm", name="z_ps")
                nc.tensor.matmul(z_ps[:], t1sb[g][:],
                                 megsb[g][:, SC * N:(SC + 1) * N],
                                 start=True, stop=True)
                x1[g] = wpool.tile([128, GH], bf16, tag="x1", bufs=BL,
                                   name="x1")
                relu_to(x1[g], z_ps, b1t, "x1", bias_col=True)

            def w2_g(g):
                t2_ps = psB.tile([128, GH], f32, tag="mm", name="t2_ps")
                nc.tensor.matmul(t2_ps[:], x1[g][:], W2c,
                                 start=True, stop=True)
                t2sb[g] = wpool.tile([128, GH], bf16, tag="t2sb", bufs=BL,
                                     name="t2sb")
                nc.scalar.copy(t2sb[g][:], t2_ps[:])

            def z2_g(g):
                z2_ps = psB.tile([128, GH], f32, tag="mm", name="z2_ps")
                nc.tensor.matmul(z2_ps[:], megsb[g][:, SC * N:(SC + 1) * N],
                                 t2sb[g][:], start=True, stop=True)
                x2[g] = wpool.tile([128, GH], bf16, tag="x2", bufs=BL,
                                   name="x2")
                relu_to(x2[g], z2_ps, b2t, "x2")

            def mp_g(g):
                mp_ps = psB.tile([128, 1], f32, tag="mm", name="mp_ps")
                nc.tensor.matmul(mp_ps[:], x2[g][:], MEAN,
                                 start=True, stop=True)
                nc.vector.tensor_copy(catT6[:, g:g + 1], mp_ps[:])

            # ---- unified software pipeline: projection/gate (steps 0..8)
            # with pooling+GCN stages for finished pairs filling PE slack ----
            STAGES = [pool_g, z1_g, w2_g, z2_g, mp_g]
            for s in range(BL + 7):
                # deep stages first (their data has been ready the longest);
                # pool (stage 0) last — its XBAR data is the freshest.
                for si in range(len(STAGES) - 1, 0, -1):
                    k2 = s - 3 - si  # == 2*k when this pair's stage is due
                    if k2 >= 0 and k2 % 2 == 0 and k2 < BL:
                        STAGES[si](k2)
                        STAGES[si](k2 + 1)
                if s < BL:
                    if s >= 1:
                        gate_sb[s - 1] = wpool.tile([128, S], bf16, tag="gt",
                                                    bufs=4, name="gate_sb")
                        nc.scalar.activation(gate_sb[s - 1][:],
                                             lg_ps[s - 1][:], AFT.Sigmoid,
                                             bias=float(br_val),
                                             scale=1.0 / SC_W)
                    yT_ps[s] = psY.tile([128, S], f32, tag="yt", name="yT_ps")
                    lg_ps[s] = psL.tile([128, S], f32, tag="lg", name="lg_ps")
                    for j in range(HC // 2):
                        lht_j = megs8[s][:, 2 * j:2 * j + 2, :]
                        nc.tensor.matmul(
                            yT_ps[s][:], W1c[:, 2 * j:2 * j + 2, :],
                            lht_j, start=(j == 0), stop=(j == HC // 2 - 1),
                            perf_mode=MPM.DoubleRow)
                        nc.tensor.matmul(
                            lg_ps[s][:], WRBc[:, 2 * j:2 * j + 2, :],
                            lht_j, start=(j == 0), stop=(j == HC // 2 - 1),
                            perf_mode=MPM.DoubleRow)
                    if s >= 1:
                        gate_into_y(s - 1)
                if s == BL:
                    gate_sb[BL - 1] = wpool.tile([128, S], bf16, tag="gt",
                                                 bufs=4, name="gate_sb")
                    nc.scalar.activation(gate_sb[BL - 1][:],
                                         lg_ps[BL - 1][:], AFT.Sigmoid,
                                         bias=float(br_val),
                                         scale=1.0 / SC_W)
                    gate_into_y(BL - 1)
                if s == 7:
                    # late head consts (Wf1/Wf2/cls): ride the sync ring in
                    # its idle window between XBAR2 and XBAR3, far from the
                    # scalar ring's meg stream
                    nc.sync.dma_start(cts[:, C8_W + 2 * C_E:],
                                      cts_d[:, C8_W + 2 * C_E:])
                if s >= 3 and (s - 3) % 2 == 0 and s - 3 < BL:
                    pool_g(s - 3)
                    pool_g(s - 2)

            # ---------- FC head over all BL graphs ----------
            h1_ps = []
            for hh in range(2):
                hp = psB.tile([128, BL], f32, tag="mm", name=f"h1_ps{hh}")
                for c in range(FC):
                    lhsT = ctB[:, C_WF1 + (c * 2 + hh) * 128:
                               C_WF1 + (c * 2 + hh + 1) * 128]
                    rhs = (ctB[:, C_CLS + c * BL:C_CLS + (c + 1) * BL]
                           if c < HC else catT6[:])
                    nc.tensor.matmul(hp[:], lhsT, rhs, start=(c == 0),
                                     stop=(c == FC - 1))
                h1_ps.append(hp)
            for hh in range(2):
                if bf1t is None:
                    nc.vector.tensor_scalar_max(h1r[:, hh, :], h1_ps[hh][:],
                                                0.0)
                else:
                    nc.vector.tensor_scalar(h1r[:, hh, :], h1_ps[hh][:],
                                            bf1t[:, hh:hh + 1], 0.0,
                                            ALU.add, ALU.max)
            out_ps = psB.tile([L, BL], f32, tag="mm", name="out_ps")
            for hh in range(2):
                nc.tensor.matmul(out_ps[:],
                                 ctB[:, C_WF2 + hh * L:C_WF2 + (hh + 1) * L],
                                 h1r[:, hh, :], start=(hh == 0),
                                 stop=(hh == 1))
            outs = cpool.tile([L, BL], f32)
            if bf2t is None:
                nc.vector.tensor_copy(outs[:], out_ps[:])
            else:
                nc.vector.tensor_scalar_add(outs[:], out_ps[:], bf2t[:])
            nc.sync.dma_start(out_d[:], outs[:])

    _split_multi_waits(nc)
    return nc


def _prepare_in_maps(inputs):
    lh = np.ascontiguousarray(np.asarray(inputs["last_hidden"], dtype=np.float32))
    submap = np.asarray(inputs["submap"]).astype(np.int64)
    edge_index = np.asarray(inputs["edge_index"]).astype(np.int64)
    assert lh.shape == (B, S, H)
    assert int(inputs.get("num_nodes", N)) == N

    wr = np.asarray(inputs["wr"], dtype=np.float32)
    br = float(np.asarray(inputs["br"], dtype=np.float32))
    W1 = np.asarray(inputs["W1"], dtype=np.float32)
    b1 = np.asarray(inputs["b1"], dtype=np.float32)
    W2 = np.asarray(inputs["W2"], dtype=np.float32)
    b2 = np.asarray(inputs["b2"], dtype=np.float32)
    Wf1 = np.asarray(inputs["Wf1"], dtype=np.float32)
    bf1 = np.asarray(inputs["bf1"], dtype=np.float32)
    Wf2 = np.asarray(inputs["Wf2"], dtype=np.float32)
    bf2 = np.asarray(inputs["bf2"], dtype=np.float32)

    # ---- host-side index prep: adjacency, degrees, counts ----
    src = edge_index[:, 0, :]
    dst = edge_index[:, 1, :]
    flat = (np.arange(B, dtype=np.int64)[:, None] * (N * N) + src * N + dst)
    A = np.bincount(flat.reshape(-1), minlength=B * N * N).astype(np.float32)
    A = A.reshape(B, N, N) + np.eye(N, dtype=np.float32)[None]
    deg = A.sum(axis=1)                      # in-degree incl self-loops
    dinv = 1.0 / np.sqrt(deg)
    ahat = A * dinv[:, :, None] * dinv[:, None, :]

    cflat = np.arange(B, dtype=np.int64)[:, None] * N + submap
    cnt = np.bincount(cflat.reshape(-1), minlength=B * N).astype(np.float32)
    invc = 1.0 / np.maximum(cnt.reshape(B, N), 1.0)

    P = (submap[:, :, None] == np.arange(N)[None, None, :]).astype(np.float32)
    P *= (invc * dinv)[:, None, :] * SC_P

    # ---- merged mega-tensor: all fp8 (lhT + scaled P'' + scaled Ahat'') ----
    lht = np.ascontiguousarray(
        lh.astype(E4M3).reshape(B, S, HC, 128).transpose(0, 3, 2, 1))
    p_r = P.astype(E4M3).reshape(B, SC, 128, N).transpose(0, 2, 1, 3)
    meg8 = np.empty((B, 128, MEGW8), dtype=E4M3)
    meg8[:, :, 0:HC * S] = lht.reshape(B, 128, HC * S)
    meg8[:, :, HC * S:HC * S + SC * N] = p_r.reshape(B, 128, SC * N)
    meg8[:, :, HC * S + SC * N:] = (SC_A * ahat).astype(E4M3)

    # ---- fp8 consts: scaled W1 + broadcast wr ----
    ct8 = np.zeros((128, C8_W), dtype=np.float32)
    ct8[:, C8_W1:C8_W1 + HC * GH] = (
        (SC_W * W1).reshape(HC, 128, GH).transpose(1, 0, 2).reshape(128, -1))
    ct8[:, C8_WRB:C8_WRB + HC * 128] = np.repeat(
        (SC_W * wr).reshape(HC, 128).T, 128, axis=1).reshape(128, HC * 128)
    # interleave into [p, hc, GH+128] layout matching the SBUF tile
    ct8_t = np.empty((128, HC, GH + 128), dtype=E4M3)
    ct8_t[:, :, 0:GH] = ct8[:, 0:HC * GH].reshape(128, HC, GH).astype(E4M3)
    ct8_t[:, :, GH:] = ct8[:, HC * GH:].reshape(128, HC, 128).astype(E4M3)
    ct8_flat = np.ascontiguousarray(ct8_t.reshape(128, -1))

    # ---- bf16 consts, cls block differs per core ----
    consts = np.zeros((128, C_W), dtype=np.float32)
    consts[:, C_W2:C_W2 + GH] = W2 / (SC_P * SC_A * SC_W)
    consts[:, C_WF1:C_WF1 + FC * 2 * 128] = (
        Wf1.reshape(FC, 128, 2, 128).transpose(1, 0, 2, 3).reshape(128, -1))
    consts[:, C_WF2:C_WF2 + 2 * L] = (
        Wf2.reshape(2, 128, L).transpose(1, 0, 2).reshape(128, 2 * L))
    consts[:, C_MEAN] = 1.0 / (SC_A * N)
    consts[:, C_IDENT:C_IDENT + 128] = np.eye(128, dtype=np.float32)

    b1b = np.ascontiguousarray(np.broadcast_to(
        (b1 * (SC_P * SC_A * SC_W)).reshape(GH, 1), (128, GH))
        .astype(np.float32))
    b2b = np.ascontiguousarray(np.broadcast_to(
        b2 * SC_A, (128, GH)).astype(np.float32))
    bf1b = np.ascontiguousarray(bf1.reshape(2, 128).T.astype(np.float32))
    bf2b = np.ascontiguousarray(bf2.reshape(L, 1).astype(np.float32))

    in_maps = []
    for i in range(NCORES):
        sl = slice(i * BL, (i + 1) * BL)
        ci = consts.copy()
        ci[:, C_CLS:C_CLS + HC * BL] = (
            lh[sl, 0, :].reshape(BL, HC, 128).transpose(2, 1, 0)
            .reshape(128, HC * BL))
        cb = ci.astype(BF16)
        cts = np.empty((128, C8_W + 2 * C_W), dtype=np.uint8)
        cts[:, 0:C8_W] = ct8_flat.view(np.uint8)
        cts[:, C8_W:] = cb.view(np.uint8)
        in_maps.append({
            "meg8": np.ascontiguousarray(meg8[sl]),
            "cts": cts.view(E4M3),
            "b1b": b1b, "b2b": b2b, "bf1b": bf1b, "bf2b": bf2b,
        })
    flags = (br, bool(np.all(b1 == 0)), bool(np.all(b2 == 0)),
             bool(np.all(bf1 == 0)), bool(np.all(bf2 == 0)))
    return in_maps, flags


def _run(inputs, trace=False):
    in_maps, flags = _prepare_in_maps(inputs)
    key = ("prog",) + flags
    if key not in _CACHE:
        _CACHE[key] = build_program(*flags)
    nc = _CACHE[key]
    res = run_bass_kernel_spmd(nc, in_maps, list(range(NCORES)), trace=trace)
    out = np.concatenate(
        [np.asarray(res.results[i]["out"]).T for i in range(NCORES)],
        axis=0).astype(np.float32)
    return out, res


def kernel(**inputs) -> np.ndarray:
    out, _ = _run(inputs, trace=False)
    return out
